# revision 10
# baseline (speedup 1.0000x reference)
"""Bass/Trainium2 kernel for nn_BlastocystAuxLoss.

Computes a masked MSE over B=16,777,216 elements:
    late stages are labels 8..15; target[s] = (s-8) * 4/7 for late stages;
    loss = sum_{s>=8} (x - target)^2 / count(s>=8)   (0.0 if count == 0)

Strategy: trivially data-parallel over 8 NeuronCores. Each core reads its
B/8 shard of blast_scores (f32) and stage_labels (i32) from HBM, computes
per-partition partial {count, sse} on-chip (DVE + ACT engines, bf16
elementwise math, f32 accumulation), and writes a [128, 2] partial-sums
tile. The final scalar reduction (8*128 partials -> sse/cnt) happens on
host in f64. No collectives needed.

Per-element identities used (s = label, x = score):
    mask  m = (s >= 8)
    target t = relu(s * 4/7 - 32/7)        (== (s-8)*4/7 clamped at 0)
    sse  += (m * (bf16(x) - t))^2          (m^2 == m)
    cnt  += m
"""

import numpy as np

B = 16777216
N_CORES = 8
SHARD = B // N_CORES  # 2,097,152
P = 128

_NC_CACHE = {}


def build(shard=SHARD, n_tiles=4):
    """Build the single-core Bass program (same SPMD program for all cores)."""
    import concourse.bacc as bacc
    import concourse.tile as tile
    from concourse import mybir

    free = shard // P
    fd = free // n_tiles
    assert fd * n_tiles * P == shard

    nc = bacc.Bacc("TRN2", target_bir_lowering=False)
    x_ext = nc.declare_dram_parameter(
        "blast_scores", [shard], mybir.dt.float32, isOutput=False
    )
    s_ext = nc.declare_dram_parameter(
        "stage_labels", [shard], mybir.dt.int32, isOutput=False
    )
    out_ext = nc.declare_dram_parameter("out", [P, 2], mybir.dt.float32, isOutput=True)

    x_v = x_ext.ap().rearrange("(p f) -> p f", p=P)
    s_v = s_ext.ap().rearrange("(p f) -> p f", p=P)

    c47 = 4.0 / 7.0  # target step; folded into the Square's input scale
    c74 = 7.0 / 4.0  # x prescale so z = 7/4*(x - t) uses integer-exact v

    f32 = mybir.dt.float32
    bf16 = mybir.dt.bfloat16
    Alu = mybir.AluOpType
    Act = mybir.ActivationFunctionType

    with tile.TileContext(nc) as tc:
        with (
            tc.tile_pool(name="io", bufs=2) as io_pool,
            tc.tile_pool(name="mid", bufs=2) as mid_pool,
            tc.tile_pool(name="acc", bufs=1) as acc_pool,
        ):
            cnt_acc = acc_pool.tile([P, n_tiles], f32)
            sse_acc = acc_pool.tile([P, n_tiles], f32)
            red = acc_pool.tile([P, 2], f32)

            for k in range(n_tiles):
                x_t = io_pool.tile([P, fd], f32, tag="x")
                s_t = io_pool.tile([P, fd], mybir.dt.int32, tag="s")
                nc.sync.dma_start(out=x_t[:], in_=x_v[:, k * fd : (k + 1) * fd])
                nc.sync.dma_start(out=s_t[:], in_=s_v[:, k * fd : (k + 1) * fd])

                u = mid_pool.tile([P, fd], bf16, tag="u")
                m = mid_pool.tile([P, fd], bf16, tag="m")
                v = mid_pool.tile([P, fd], bf16, tag="v")
                z = mid_pool.tile([P, fd], bf16, tag="z")
                zm = mid_pool.tile([P, fd], bf16, tag="zm")
                sq = mid_pool.tile([P, fd], bf16, tag="sq")

                # DVE (2x perf modes): u = min(s-7, 1); v = max(s-8, 0)
                nc.vector.tensor_scalar(u[:], s_t[:], 7, 1, Alu.subtract, Alu.min)
                nc.vector.tensor_scalar(v[:], s_t[:], 8, 0, Alu.subtract, Alu.max)
                # ACT: mask m = relu(u) in {0,1}; accumulate count for free
                nc.scalar.activation(
                    m[:], u[:], Act.Relu, accum_out=cnt_acc[:, k : k + 1]
                )
                # DVE: z = 7/4*x - v  (== 7/4*(x - target) since v = 7/4*t)
                nc.vector.scalar_tensor_tensor(
                    z[:], x_t[:], c74, v[:], Alu.mult, Alu.subtract
                )
                nc.vector.tensor_tensor(zm[:], z[:], m[:], Alu.mult)
                # ACT: sse += (4/7 * zm)^2 over masked elements
                nc.scalar.activation(
                    sq[:], zm[:], Act.Square, scale=c47,
                    accum_out=sse_acc[:, k : k + 1],
                )

            nc.vector.reduce_sum(red[:, 0:1], cnt_acc[:], axis=mybir.AxisListType.X)
            nc.vector.reduce_sum(red[:, 1:2], sse_acc[:], axis=mybir.AxisListType.X)
            nc.sync.dma_start(out=out_ext.ap()[:, :], in_=red[:])

    nc.finalize()
    return nc


def run(x, s, **spmd_kwargs):
    """Shard, run on 8 cores, host-reduce. Returns (loss, BassKernelResults)."""
    from concourse.bass_utils import run_bass_kernel_spmd

    if "nc" not in _NC_CACHE:
        _NC_CACHE["nc"] = build()
    nc = _NC_CACHE["nc"]

    in_maps = [
        {
            "blast_scores": x[i * SHARD : (i + 1) * SHARD],
            "stage_labels": s[i * SHARD : (i + 1) * SHARD],
        }
        for i in range(N_CORES)
    ]
    res = run_bass_kernel_spmd(nc, in_maps, core_ids=list(range(N_CORES)), **spmd_kwargs)

    tot = np.zeros(2, dtype=np.float64)
    for r in res.results:
        tot += r["out"].astype(np.float64).sum(axis=0)
    cnt, sse = tot[0], tot[1]
    val = sse / max(cnt, 1.0) if cnt > 0 else 0.0
    return np.asarray(val, dtype=np.float32), res


def kernel(**inputs):
    x = np.ascontiguousarray(np.asarray(inputs["blast_scores"], dtype=np.float32))
    s = np.ascontiguousarray(np.asarray(inputs["stage_labels"], dtype=np.int32))
    assert x.shape == (B,) and s.shape == (B,)
    return run(x, s)[0]
